# revision 14
# baseline (speedup 1.0000x reference)
"""Trainium2 Bass kernel for nn_Encoder_M (dense_cnn).

Network (eval mode):
  x1 [B,1,120,4,2] -> conv3d(1->1, k=(7,3,1), pad=(3,1,0)) + b1 -> relu
     -> maxpool_d(k=4) (vals + flat argmax indices)
     -> conv3d(k=(7,3,1), pad=(3,1,0)) + b2 -> relu
     -> maxpool_d(k=3) (vals + indices)
     -> flatten [B,80] -> fcm1 (80->10) -> ELU
  x2 [B,1] -> sigmoid -> concat -> [B,11] -> fc (11->100) -> /sqrt(1+1e-5)
  outputs: (z, aug_xm=x1, aug_x_sd=x2, pool1_idx, pool2_idx)

Strategy: pure data-parallel over 8 NeuronCores (batch sharded 4096/core).
Identity outputs (aug_xm, aug_x_sd) never touch the device.  On device,
convolutions run on TensorE as "x-stationary" matmuls that contract over the
feature axis and emit batch-major output directly into PSUM; pooling (max +
first-occurrence argmax) is a compare/select tournament on VectorE/GpSimdE;
ScalarE applies bias+relu while evicting PSUM; the two tiny FC layers are
matmuls over transposed activations.  Host pre-transposes x1 to feature-major
per shard so every DMA is contiguous.
"""

import numpy as np
from contextlib import ExitStack

import concourse.bass as bass
import concourse.tile as tile
from concourse import bacc, mybir
from concourse.bass_utils import run_bass_kernel_spmd

F32 = mybir.dt.float32
I32 = mybir.dt.int32
ALU = mybir.AluOpType
ACTF = mybir.ActivationFunctionType

B, D, H, W = 32768, 120, 4, 2
HWF = H * W  # 8
NCORES = 8
NC_B = B // NCORES          # 4096 samples per core
TILE_B = 128                # samples per tile (partition dim)
GROUP = 4                   # tiles per output-staging DMA
BN_SCALE = float(1.0 / np.sqrt(1.0 + 1e-5))

# conv1 h'-windows per h_in: (lo, hi) inclusive.  The hin=1,2 matmuls are
# padded to the full h' range so the start-of-group matmul covers the whole
# PSUM bank region (keeps accumulation groups bank-aligned); hin=0,3 stay
# windowed and land entirely inside the already-initialized region.
H_WIN = [(max(0, h - 1), min(H - 1, h + 1)) for h in range(H)]
A1_RANGE = [H_WIN[0], (0, 3), (0, 3), H_WIN[3]]
A1_WIDTHS = [(hi - lo + 1) * D for lo, hi in A1_RANGE]        # [240,480,480,240]
A1_OFFS = np.concatenate([[0], np.cumsum(A1_WIDTHS)]).tolist()  # -> 1440 total
A1_COLS = A1_OFFS[-1]
CONV1_ORDER = [1, 2, 0, 3]


def _build_a1(w):
    """w: [7,3] (kd, kh). Returns [120, A1_COLS] f32, cols ordered (h', d')."""
    a1 = np.zeros((D, A1_COLS), np.float32)
    din = np.arange(D)[:, None]
    dp = np.arange(D)[None, :]
    kd = din - dp + 3
    valid = (kd >= 0) & (kd < 7)
    kdc = np.clip(kd, 0, 6)
    for hin in range(H):
        lo, hi = A1_RANGE[hin]
        wlo, whi = H_WIN[hin]
        for i, hp in enumerate(range(lo, hi + 1)):
            if not (wlo <= hp <= whi):
                continue  # padded zero block
            kh = hin - hp + 1
            blk = np.where(valid, w[kdc, kh], 0.0)
            c0 = A1_OFFS[hin] + i * D
            a1[:, c0:c0 + D] = blk
    return a1


def _build_a2(w):
    """w: [7,3]. Returns [120,120]: rows (d2_in*4+h_in), cols (h'*30+d3')."""
    D2, D3 = 30, 10 * 3  # conv2 operates on depth-30 input
    a2 = np.zeros((120, 120), np.float32)
    for d2 in range(30):
        for hin in range(H):
            r = d2 * 4 + hin
            for hp in range(H):
                kh = hin - hp + 1
                if not (0 <= kh < 3):
                    continue
                for d3 in range(30):
                    kd = d2 - d3 + 3
                    if 0 <= kd < 7:
                        a2[r, hp * 30 + d3] = w[kd, kh]
    return a2


def _build_nc(ntiles):
    """Build + compile the SPMD Bass program for `ntiles` tiles of 128 samples."""
    nb = ntiles * TILE_B
    nc = bacc.Bacc("TRN2", target_bir_lowering=False, debug=False,
                   num_devices=NCORES)

    xt_d = nc.dram_tensor("xt", [D, nb, HWF], F32, kind="ExternalInput")
    x2t_d = nc.dram_tensor("x2t", [TILE_B, ntiles], F32, kind="ExternalInput")
    a1_d = nc.dram_tensor("a1", [D, A1_COLS], F32, kind="ExternalInput")
    a2_d = nc.dram_tensor("a2", [120, 120], F32, kind="ExternalInput")
    fw1_d = nc.dram_tensor("fw1", [81, 10], F32, kind="ExternalInput")
    fw2_d = nc.dram_tensor("fw2", [12, 100], F32, kind="ExternalInput")
    id_d = nc.dram_tensor("ident", [128, 128], F32, kind="ExternalInput")
    b1_d = nc.dram_tensor("b1r", [128, 1], F32, kind="ExternalInput")
    b2_d = nc.dram_tensor("b2r", [128, 1], F32, kind="ExternalInput")

    z_d = nc.dram_tensor("z", [nb, 100], F32, kind="ExternalOutput")
    i1_d = nc.dram_tensor("idx1", [nb, 240], I32, kind="ExternalOutput")
    i2_d = nc.dram_tensor("idx2", [nb, 80], I32, kind="ExternalOutput")

    with tile.TileContext(nc) as tc:
        _kernel(tc, ntiles, xt_d, x2t_d, a1_d, a2_d, fw1_d, fw2_d, id_d,
                b1_d, b2_d, z_d, i1_d, i2_d)
    nc.compile()
    return nc


def _kernel(tc, ntiles, xt_d, x2t_d, a1_d, a2_d, fw1_d, fw2_d, id_d,
            b1_d, b2_d, z_d, i1_d, i2_d):
    nc = tc.nc
    with ExitStack() as ctx:
        cpool = ctx.enter_context(tc.tile_pool(name="const", bufs=1))
        xpool = ctx.enter_context(tc.tile_pool(name="x", bufs=3))
        vpool = ctx.enter_context(tc.tile_pool(name="v", bufs=2))
        spool = ctx.enter_context(tc.tile_pool(name="scratch", bufs=2))
        opool = ctx.enter_context(tc.tile_pool(name="outs", bufs=2))
        p_c1 = ctx.enter_context(
            tc.tile_pool(name="p_c1", bufs=2, space=bass.MemorySpace.PSUM))
        p_tw = ctx.enter_context(
            tc.tile_pool(name="p_tw", bufs=1, space=bass.MemorySpace.PSUM))
        p_c2 = ctx.enter_context(
            tc.tile_pool(name="p_c2", bufs=1, space=bass.MemorySpace.PSUM))
        p_sm = ctx.enter_context(
            tc.tile_pool(name="p_sm", bufs=1, space=bass.MemorySpace.PSUM))

        # ---- resident constants ----
        a1 = cpool.tile([D, A1_COLS], F32)
        nc.sync.dma_start(a1[:], a1_d.ap())
        a2 = cpool.tile([120, 120], F32)
        nc.sync.dma_start(a2[:], a2_d.ap())
        fw1 = cpool.tile([81, 10], F32)
        nc.sync.dma_start(fw1[:], fw1_d.ap())
        fw2 = cpool.tile([12, 100], F32)
        nc.sync.dma_start(fw2[:], fw2_d.ap())
        ident = cpool.tile([128, 128], F32)
        nc.sync.dma_start(ident[:], id_d.ap())
        b1r = cpool.tile([128, 1], F32)
        nc.sync.dma_start(b1r[:], b1_d.ap())
        b2r = cpool.tile([128, 1], F32)
        nc.sync.dma_start(b2r[:], b2_d.ap())
        x2t = cpool.tile([TILE_B, ntiles], F32)
        nc.sync.dma_start(x2t[:], x2t_d.ap())

        base1i = cpool.tile([128, 240], I32)  # 32*d2 + 2*h + w
        nc.gpsimd.iota(base1i[:], pattern=[[32, 30], [2, 4], [1, 2]],
                       base=0, channel_multiplier=0)
        base1 = cpool.tile([128, 240], F32)
        nc.vector.tensor_copy(base1[:], base1i[:])
        base2i = cpool.tile([128, 80], I32)   # 24*d3 + 2*h + w
        nc.gpsimd.iota(base2i[:], pattern=[[24, 10], [2, 4], [1, 2]],
                       base=0, channel_multiplier=0)
        base2 = cpool.tile([128, 80], F32)
        nc.vector.tensor_copy(base2[:], base2i[:])
        two = cpool.tile([128, 80], F32)
        nc.gpsimd.memset(two[:], 2.0)

        ngroups = ntiles // GROUP
        for g in range(ngroups):
            z_st = opool.tile([TILE_B, GROUP, 100], F32, tag="z_st")
            i1_st = opool.tile([TILE_B, GROUP, 240], F32, tag="i1_st")
            i2_st = opool.tile([TILE_B, GROUP, 80], F32, tag="i2_st")
            for tt in range(GROUP):
                t = g * GROUP + tt
                _tile_body(nc, tc, t, tt, xt_d,
                           a1, a2, fw1, fw2, ident, b1r, b2r, x2t,
                           base1, base2, two,
                           xpool, vpool, spool, p_c1, p_tw, p_c2, p_sm,
                           z_st, i1_st, i2_st)
            # staged output DMAs: [128, GROUP, f] -> rows [g*512, (g+1)*512)
            zv = z_d.ap().rearrange("(g tt p) f -> g p tt f",
                                    g=ngroups, tt=GROUP, p=TILE_B)[g]
            nc.sync.dma_start(zv, z_st[:])
            i1v = i1_d.ap().rearrange("(g tt p) f -> g p tt f",
                                      g=ngroups, tt=GROUP, p=TILE_B)[g]
            nc.gpsimd.dma_start(i1v, i1_st[:])  # SWDGE casts f32->int32
            i2v = i2_d.ap().rearrange("(g tt p) f -> g p tt f",
                                      g=ngroups, tt=GROUP, p=TILE_B)[g]
            nc.gpsimd.dma_start(i2v, i2_st[:])  # SWDGE casts f32->int32


def _tile_body(nc, tc, t, tt, xt_d, a1, a2, fw1, fw2, ident, b1r, b2r, x2t,
               base1, base2, two, xpool, vpool, spool,
               p_c1, p_tw, p_c2, p_sm, z_st, i1_st, i2_st):
    f32, i32 = F32, I32

    # ---- load x tile, feature-major [120, 128, 8] ----
    xt = xpool.tile([D, TILE_B, HWF], f32, tag="xt")
    nc.sync.dma_start(xt[:], xt_d.ap()[:, t * TILE_B:(t + 1) * TILE_B, :])

    # ---- conv1: 8 matmuls, batch-major out [128, (w:512)(h':120)(d':1)] ----
    c1 = p_c1.tile([128, 1024], f32, tag="c1")
    for w in range(W):
        for i, hin in enumerate(CONV1_ORDER):
            lo, hi = A1_RANGE[hin]
            ncols = A1_WIDTHS[hin]
            lhsT = xt[:, :, hin * W + w]                      # [120,128] stride 8
            rhs = a1[:, A1_OFFS[hin]:A1_OFFS[hin] + ncols]    # [120,ncols]
            out = c1[:, w * 512 + lo * D: w * 512 + (hi + 1) * D]
            nc.tensor.matmul(out, lhsT, rhs, start=(i == 0), stop=(i == 3),
                             skip_group_check=True)

    # ---- bias + relu eviction, r-major layout [128, (r:240)(d2:8)(h:2)(w:1)] ----
    v1 = vpool.tile([128, 960], f32, tag="v1")
    for w in range(W):
        c1v = c1[:, w * 512:w * 512 + 480].rearrange(
            "p (h d2 r) -> p h d2 r", h=4, d2=30, r=4)
        v1v = v1[:].rearrange("p (r d2 h w) -> p w h d2 r",
                              r=4, d2=30, h=4, w=2)[:, w]
        nc.scalar.activation(v1v, c1v, ACTF.Relu, bias=b1r[:], scale=1.0)

    def vs(r):  # v1 slice r: [128, 240] contiguous (d2,h,w)
        return v1[:, r * 240:(r + 1) * 240]

    # ---- pool1: max + first-occurrence argmax over r in {0..3} ----
    m01 = spool.tile([128, 240], f32, tag="m01")
    nc.vector.tensor_tensor(m01[:], vs(0), vs(1), ALU.max)
    m23 = spool.tile([128, 240], f32, tag="m23")
    nc.vector.tensor_tensor(m23[:], vs(2), vs(3), ALU.max)
    m1 = spool.tile([128, 240], f32, tag="m1")  # layout (w:120)(d2:4)(h:1)
    m1v = m1[:].rearrange("p (w d2 h) -> p d2 h w", w=2, d2=30, h=4)
    m1in0 = m01[:].rearrange("p (d2 h w) -> p d2 h w", d2=30, h=4, w=2)
    m1in1 = m23[:].rearrange("p (d2 h w) -> p d2 h w", d2=30, h=4, w=2)
    nc.vector.tensor_tensor(m1v, m1in0, m1in1, ALU.max)

    s10 = spool.tile([128, 240], f32, tag="s10")
    nc.gpsimd.tensor_tensor(s10[:], vs(1), vs(0), ALU.subtract)
    s32 = spool.tile([128, 240], f32, tag="s32")
    nc.gpsimd.tensor_tensor(s32[:], vs(3), vs(2), ALU.subtract)
    g1 = spool.tile([128, 240], f32, tag="g1")
    nc.vector.tensor_scalar(g1[:], s10[:], 0.0, None, op0=ALU.is_gt)
    i23 = spool.tile([128, 240], f32, tag="i23")
    nc.vector.tensor_scalar(i23[:], s32[:], 0.0, 2.0,
                            op0=ALU.is_gt, op1=ALU.add)
    g23 = spool.tile([128, 240], i32, tag="g23")
    nc.vector.tensor_tensor(g23[:], m23[:], m01[:], ALU.is_gt)
    off = spool.tile([128, 240], f32, tag="off")
    nc.scalar.copy(off[:], g1[:])
    nc.vector.copy_predicated(off[:], g23[:], i23[:])
    nc.vector.scalar_tensor_tensor(i1_st[:, tt, :], off[:], 8.0, base1[:],
                                   op0=ALU.mult, op1=ALU.add)

    # ---- conv2 lhsT: transpose m1 per w -> [120=(d2*4+h), 128] ----
    twp = p_tw.tile([120, 256], f32, tag="twp")
    for w in range(W):
        nc.tensor.matmul(twp[:, w * 128:(w + 1) * 128],
                         m1[:, w * 120:(w + 1) * 120], ident[:],
                         is_transpose=True, start=(w == 0), stop=(w == 1),
                         skip_group_check=True)
    lh2 = spool.tile([120, 256], f32, tag="lh2")
    nc.vector.tensor_copy(lh2[:], twp[:])

    # ---- conv2: 2 matmuls (K=(d2,h)-stacked), out [128, (w:128)(h':30)(d3':1)] ----
    c2 = p_c2.tile([128, 256], f32, tag="c2")
    for w in range(W):
        nc.tensor.matmul(c2[:, w * 128:w * 128 + 120],
                         lh2[:, w * 128:(w + 1) * 128], a2[:],
                         start=(w == 0), stop=(w == 1), skip_group_check=True)

    # ---- bias + relu eviction -> v2 [128, (r:80)(d3:8)(h:2)(w:1)] ----
    v2 = vpool.tile([128, 240], f32, tag="v2")
    for w in range(W):
        c2v = c2[:, w * 128:w * 128 + 120].rearrange(
            "p (h d3 r) -> p h d3 r", h=4, d3=10, r=3)
        v2v = v2[:].rearrange("p (r d3 h w) -> p w h d3 r",
                              r=3, d3=10, h=4, w=2)[:, w]
        nc.scalar.activation(v2v, c2v, ACTF.Relu, bias=b2r[:], scale=1.0)

    def us(r):  # v2 slice r: [128, 80] contiguous (d3,h,w)
        return v2[:, r * 80:(r + 1) * 80]

    # ---- pool2: max + argmax over r in {0..2}; xm [128, 81] w/ ones col ----
    n01 = spool.tile([128, 80], f32, tag="n01")
    nc.vector.tensor_tensor(n01[:], us(0), us(1), ALU.max)
    xm = spool.tile([128, 81], f32, tag="xm")
    nc.vector.tensor_tensor(xm[:, 0:80], n01[:], us(2), ALU.max)
    nc.vector.memset(xm[:, 80:81], 1.0)

    t10 = spool.tile([128, 80], f32, tag="t10")
    nc.gpsimd.tensor_tensor(t10[:], us(1), us(0), ALU.subtract)
    h1 = spool.tile([128, 80], f32, tag="h1")
    nc.vector.tensor_scalar(h1[:], t10[:], 0.0, None, op0=ALU.is_gt)
    h2 = spool.tile([128, 80], i32, tag="h2")
    nc.vector.tensor_tensor(h2[:], us(2), n01[:], ALU.is_gt)
    off2 = spool.tile([128, 80], f32, tag="off2")
    nc.scalar.copy(off2[:], h1[:])
    nc.vector.copy_predicated(off2[:], h2[:], two[:])
    nc.vector.scalar_tensor_tensor(i2_st[:, tt, :], off2[:], 8.0, base2[:],
                                   op0=ALU.mult, op1=ALU.add)

    # ---- small FC chain: transposes in one PSUM bank, FC outs in another ----
    smT = p_sm.tile([128, 512], f32, tag="smT")
    smF = p_sm.tile([128, 512], f32, tag="smF")
    # xm^T [81, 128]
    nc.tensor.matmul(smT[0:81, 0:128], xm[:], ident[:],
                     is_transpose=True, start=True, stop=False,
                     skip_group_check=True)
    xmT = spool.tile([81, 128], f32, tag="xmT")
    nc.scalar.copy(xmT[:], smT[0:81, 0:128])
    # fcm1: u = xmT.T @ fw1 -> [128, 10]
    u1 = smF[:, 0:10]
    nc.tensor.matmul(u1, xmT[:], fw1[:], start=True, stop=False,
                     skip_group_check=True)
    # ELU -> xcat[:, :10]; sigmoid(x2) -> xcat[:, 10]; ones -> xcat[:, 11]
    xcat = spool.tile([128, 12], f32, tag="xcat")
    tmin = spool.tile([128, 10], f32, tag="tmin")
    nc.vector.tensor_scalar_min(tmin[:], u1, 0.0)
    texp = spool.tile([128, 10], f32, tag="texp")
    nc.scalar.activation(texp[:], tmin[:], ACTF.Exp)
    nc.vector.tensor_scalar_add(xcat[:, 0:10], texp[:], -1.0)
    gpos = spool.tile([128, 10], i32, tag="gpos")
    nc.vector.tensor_scalar(gpos[:], u1, 0.0, None, op0=ALU.is_gt)
    nc.vector.copy_predicated(xcat[:, 0:10], gpos[:], u1)
    nc.scalar.activation(xcat[:, 10:11], x2t[:, t:t + 1], ACTF.Sigmoid)
    nc.vector.memset(xcat[:, 11:12], 1.0)
    # xcat^T [12, 128]
    nc.tensor.matmul(smT[0:12, 128:256], xcat[:], ident[:],
                     is_transpose=True, start=False, stop=True,
                     skip_group_check=True)
    xcT = spool.tile([12, 128], f32, tag="xcT")
    nc.scalar.copy(xcT[:], smT[0:12, 128:256])
    # fc: z = xcT.T @ fw2 -> [128, 100]; scale during eviction
    zp = smF[:, 128:228]
    nc.tensor.matmul(zp, xcT[:], fw2[:], start=False, stop=True,
                     skip_group_check=True)
    nc.scalar.mul(z_st[:, tt, :], zp, BN_SCALE)


# ------------------------- host side -------------------------

_NC_CACHE = {}


def _get_nc(ntiles):
    if ntiles not in _NC_CACHE:
        _NC_CACHE[ntiles] = _build_nc(ntiles)
    return _NC_CACHE[ntiles]


def _host_consts(w1, b1, w2, b2, fcm1_w, fcm1_b, fc_w, fc_b):
    a1 = _build_a1(np.asarray(w1, np.float32)[0, 0, :, :, 0])
    a2 = _build_a2(np.asarray(w2, np.float32)[0, 0, :, :, 0])
    fw1 = np.zeros((81, 10), np.float32)
    fw1[:80] = np.asarray(fcm1_w, np.float32).T
    fw1[80] = np.asarray(fcm1_b, np.float32)
    fw2 = np.zeros((12, 100), np.float32)
    fw2[:11] = np.asarray(fc_w, np.float32).T
    fw2[11] = np.asarray(fc_b, np.float32)
    ident = np.eye(128, dtype=np.float32)
    b1r = np.broadcast_to(np.asarray(b1, np.float32).reshape(1, 1),
                          (128, 1)).copy()
    b2r = np.broadcast_to(np.asarray(b2, np.float32).reshape(1, 1),
                          (128, 1)).copy()
    return dict(a1=a1, a2=a2, fw1=fw1, fw2=fw2, ident=ident, b1r=b1r, b2r=b2r)


def kernel(x1, x2, shifts, nonzero_mask_xm, w1, b1, w2, b2,
           fcm1_w, fcm1_b, fc_w, fc_b):
    x1 = np.asarray(x1)
    x2 = np.asarray(x2)
    nb_total = x1.shape[0]
    nb = nb_total // NCORES
    ntiles = nb // TILE_B
    assert ntiles % GROUP == 0

    consts = _host_consts(w1, b1, w2, b2, fcm1_w, fcm1_b, fc_w, fc_b)

    x1f = np.ascontiguousarray(x1, dtype=np.float32).reshape(nb_total, D, HWF)
    x2f = np.ascontiguousarray(x2, dtype=np.float32).reshape(nb_total)

    in_maps = []
    for c in range(NCORES):
        sl = slice(c * nb, (c + 1) * nb)
        xt = np.ascontiguousarray(x1f[sl].transpose(1, 0, 2))  # [120, nb, 8]
        x2t = np.ascontiguousarray(
            x2f[sl].reshape(ntiles, TILE_B).T)                 # [128, ntiles]
        in_maps.append(dict(xt=xt, x2t=x2t, **consts))

    nc = _get_nc(ntiles)
    r = run_bass_kernel_spmd(nc, in_maps, list(range(NCORES)))
    globals()["LAST_EXEC_NS"] = r.exec_time_ns
    res = r.results

    z = np.concatenate([res[c]["z"] for c in range(NCORES)], axis=0)
    i1 = np.concatenate([res[c]["idx1"] for c in range(NCORES)], axis=0)
    i2 = np.concatenate([res[c]["idx2"] for c in range(NCORES)], axis=0)

    z = np.ascontiguousarray(z, np.float32)
    pool1_idx = i1.astype(np.int32).reshape(nb_total, 1, 30, 4, 2)
    pool2_idx = i2.astype(np.int32).reshape(nb_total, 1, 10, 4, 2)
    aug_xm = np.asarray(x1, np.float32)
    aug_x_sd = np.asarray(x2, np.float32)
    return z, aug_xm, aug_x_sd, pool1_idx, pool2_idx
